# revision 22
# baseline (speedup 1.0000x reference)
"""Trainium2 Bass kernel for a 2-layer GCN + global mean pool + MLP head.

Strategy (8 NeuronCores, SPMD), v4:
  - Nodes (and incident edges grouped by destination window) sharded across
    8 cores; each core owns N/8 destination nodes.
  - Layer 1 needs NO dynamic gather: gather indices are static (edge_index
    is a host input), so the host materializes per-edge source features
    (x[src] * dinv[src] * dinv[dst], 14 cols padded to 16, fp8) directly in
    gather-slot layout.  The kernel streams the slab with large static DMAs
    and aggregates per destination window with one-hot DoubleRow-fp8
    matmuls, producing aggT = (A_norm x)^T with no transpose step.
  - Layer 2's table h2t = (h1 @ W2) * dinv is computed on-chip in fp8 and
    AllGather-ed in FOUR chunks into two 3-D shared tables (each < 32768
    rows so int16 gather indices address it with a single base); chunks
    fire at group boundaries so most collective time overlaps phase A.
    Per-edge dma_gather spreads over FOUR SWDGE queues (two per table,
    tile-granular balanced split; descriptor generation runs concurrently
    per queue on separate GpSimd core pairs).
  - selu(z) = lam*Relu(z) - lam*alpha*Relu(1 - exp(z)): 3 scalar acts +
    one DVE subtract.
  - Batch ids are sorted, so each core's nodes cover a contiguous slice of
    graphs (boundary graphs shared with neighbours).  Pooling runs per
    core over its local graph range; a 16-row AllGather exchanges the two
    boundary partials, each core runs the MLP head for the graphs it owns,
    and the host assembles the output slices.
"""

import os
import numpy as np
import ml_dtypes

import concourse.bacc as bacc
import concourse.bass as bass
import concourse.mybir as mybir
import concourse.tile as tile
from concourse.bass_utils import run_bass_kernel_spmd

F32 = mybir.dt.float32
F8 = mybir.dt.float8e4
BF16 = mybir.dt.bfloat16
I16 = mybir.dt.int16
AF = mybir.ActivationFunctionType
OP = mybir.AluOpType
DR = mybir.MatmulPerfMode.DoubleRow
NPBF16 = ml_dtypes.bfloat16
NPF8 = ml_dtypes.float8_e4m3

SELU_LAM = 1.0507009873554805
SELU_ALPHA = 1.6732632423543772
SELU_LA = SELU_LAM * SELU_ALPHA

P = 128
NCORES = 8
NQ = 4          # SWDGE queues
WA_GRPS = 9     # table A = first WA_GRPS * grp windows (27 for grp=3)
CHUNK_GRPS = (5, 4, 4, 4)   # collective chunks in groups (A1 A2 | B1 B2)


def share_of(T, k, qpar):
    hi, lo = -(-T // 2), T // 2
    if k % 2 == 0:
        return hi if qpar == 0 else lo
    return lo if qpar == 0 else hi


class Cfg:
    def __init__(self, n_nodes, n_graphs, d_in, d_hid, d_fc, n_cls, th_cw, grp,
                 ngmax):
        assert n_nodes % NCORES == 0
        self.N = n_nodes
        self.G = n_graphs
        self.DIN = d_in
        self.DH = d_hid
        self.DFC = d_fc
        self.NCLS = n_cls
        self.NSH = n_nodes // NCORES
        self.W = -(-self.NSH // P)
        self.GRP = grp
        self.NGMAX = ngmax
        self.WA = WA_GRPS * grp
        self.ROWSA = min(self.WA * P, self.NSH)
        self.ROWSB = self.NSH - self.ROWSA
        self.NA = NCORES * self.ROWSA
        self.NB = NCORES * self.ROWSB
        assert self.NA < 32768 and self.NB < 32768
        self.chunks = chunk_geom(grp, self.NSH)
        self.groups = []
        w = 0
        while w < self.W:
            wg = min(grp, self.W - w)
            self.groups.append((w, wg))
            w += wg
        self.TH = []
        for (w0, wg) in self.groups:
            th = []
            for h in range(2):
                t = max(-(-int(th_cw[c, w, h]) // P)
                        for c in range(NCORES) for w in range(w0, w0 + wg))
                th.append(max(t, 1))
            self.TH.append(th)
        self.grp_slot_base = []
        self.grp_q_off = []
        self.grp_q_n = []
        self.grp_idx_col = []
        self.win_q_off = []
        slot = 0
        col = 0
        for g, (w0, wg) in enumerate(self.groups):
            self.grp_slot_base.append(slot)
            qoffs, qns, icols, woffs = [], [], [], []
            off = 0
            for q in range(NQ):
                h, par = q // 2, q % 2
                qoffs.append(off)
                icols.append(col)
                wo = []
                for k in range(wg):
                    wo.append(off)
                    off += share_of(self.TH[g][h], k, par)
                woffs.append(wo)
                nq = off - qoffs[-1]
                qns.append(nq)
                col += nq * 8
            self.grp_q_off.append(qoffs)
            self.grp_q_n.append(qns)
            self.grp_idx_col.append(icols)
            self.win_q_off.append(woffs)
            slot += off
        self.SLOT_TOT = slot
        self.IDX_COLS = col
        self.grp_nslots = [sum(self.grp_q_n[g]) for g in range(len(self.groups))]
        # collective chunk boundaries in windows
        cb = np.cumsum([0] + [c * grp for c in CHUNK_GRPS])
        assert cb[-1] == self.W or cb[-1] == len(self.groups) * grp
        self.chunk_end_w = [min(int(x), self.W) for x in cb[1:]]
        assert self.chunk_end_w[1] == self.WA


def _win_runs(cfg, g, k):
    runs = []
    for q in range(NQ):
        h, par = q // 2, q % 2
        n = share_of(cfg.TH[g][h], k, par)
        runs.append((cfg.win_q_off[g][q][k], n))
    return runs


def chunk_geom(grp, nsh):
    """Chunk row ranges per core: [(half, row_lo, row_hi, base_row), ...]
    with row_lo/row_hi relative to the half's shard.  The shared tables are
    laid out chunk-major (chunk, core, rows) so every chunk's AllGather
    output is a contiguous region."""
    ends = np.cumsum([c * grp for c in CHUNK_GRPS])
    ends = [min(int(e) * P, nsh) for e in ends]
    rowsa = ends[1]
    out = []
    base_a = base_b = 0
    lo = 0
    for ci, hi in enumerate(ends):
        if ci < 2:
            out.append((0, lo, hi, base_a))
            base_a += NCORES * (hi - lo)
        else:
            out.append((1, lo - rowsa, hi - rowsa, base_b))
            base_b += NCORES * (hi - lo)
        lo = hi
    return out


def edge_half_map(cfg, s):
    r = s // cfg.NSH
    i = s % cfg.NSH
    h = np.zeros(len(s), np.int64)
    rel = np.zeros(len(s), np.int64)
    for (half, lo, hi, base) in cfg.chunks:
        alo = lo + (cfg.ROWSA if half else 0)
        ahi = hi + (cfg.ROWSA if half else 0)
        m = (i >= alo) & (i < ahi)
        nrows = hi - lo
        h[m] = half
        rel[m] = base + r[m] * nrows + (i[m] - alo)
    return h, rel


def sort_edges(inputs, n_nodes):
    ei = np.asarray(inputs["edge_index"], np.int64)
    N = n_nodes
    src = np.concatenate([ei[0], np.arange(N, dtype=np.int64)])
    dst = np.concatenate([ei[1], np.arange(N, dtype=np.int64)])
    order = np.argsort(dst, kind="stable")
    return src[order], dst[order]


def compute_tile_budget(cfg_like, s, d, n_nodes, nsh):
    W = -(-nsh // P)
    bounds = [c * nsh + w * P for c in range(NCORES) for w in range(W)] + [n_nodes]
    cut = np.searchsorted(d, np.asarray(bounds))
    h, _ = edge_half_map(cfg_like, s)
    th_cw = np.zeros((NCORES, W, 2), np.int64)
    for i in range(NCORES * W):
        hs = h[cut[i]: cut[i + 1]]
        th_cw[i // W, i % W, 0] = int((hs == 0).sum())
        th_cw[i // W, i % W, 1] = int((hs == 1).sum())
    return th_cw, cut


class CfgLike:
    def __init__(self, n_nodes, grp):
        self.N = n_nodes
        self.NSH = n_nodes // NCORES
        self.W = -(-self.NSH // P)
        self.WA = WA_GRPS * grp
        self.ROWSA = min(self.WA * P, self.NSH)
        self.ROWSB = self.NSH - self.ROWSA
        self.chunks = chunk_geom(grp, self.NSH)


def graph_ranges(batch, n_nodes, n_graphs):
    """Per-core local graph ranges (batch is sorted)."""
    nsh = n_nodes // NCORES
    glo, ghi = [], []
    for c in range(NCORES):
        glo.append(int(batch[c * nsh]))
        ghi.append(int(batch[(c + 1) * nsh - 1]))
    ng = [ghi[c] - glo[c] + 1 for c in range(NCORES)]
    return glo, ghi, ng


def host_prep(inputs, cfg, s, d, cut):
    N, G = cfg.N, cfg.G
    x = np.asarray(inputs["x"], np.float64)
    batch = np.asarray(inputs["batch"], np.int64)

    deg = np.bincount(d, minlength=N).astype(np.float64)
    dinv = 1.0 / np.sqrt(np.maximum(deg, 1.0))
    norm = dinv[s] * dinv[d]
    hmap, rel = edge_half_map(cfg, s)

    W = cfg.W
    cnt = np.bincount(batch, minlength=G).astype(np.float32)
    cntinv = (1.0 / np.maximum(cnt, 1.0)).astype(np.float32)
    glo, ghi, ng = graph_ranges(batch, N, G)
    NGM = cfg.NGMAX

    per_core = []
    for c in range(NCORES):
        atab = np.zeros((P, cfg.SLOT_TOT, 16), NPF8)
        oht = np.zeros((P, cfg.SLOT_TOT, P), NPF8)
        iflat = np.zeros((cfg.SLOT_TOT, P), np.int16)
        for g, (w0, wg) in enumerate(cfg.groups):
            base = cfg.grp_slot_base[g]
            for k in range(wg):
                w = w0 + k
                a, b = cut[c * W + w], cut[c * W + w + 1]
                sw, dw = s[a:b], d[a:b]
                nw, rw, hw = norm[a:b], rel[a:b], hmap[a:b]
                dl = (dw - (c * cfg.NSH + w * P)).astype(np.int64)
                for h in range(2):
                    m = hw == h
                    se, de, ne, re = sw[m], dl[m], nw[m], rw[m]
                    n = len(se)
                    s0 = share_of(cfg.TH[g][h], k, 0) * P
                    for par in range(2):
                        q = h * 2 + par
                        t0 = base + cfg.win_q_off[g][q][k]
                        seq = slice(0, min(n, s0)) if par == 0 else slice(min(n, s0), n)
                        sq, dq, nq_, rq = se[seq], de[seq], ne[seq], re[seq]
                        j = np.arange(len(sq))
                        slotq = t0 + j // P
                        lane = j % P
                        atab[lane, slotq, : cfg.DIN] = (
                            x[sq] * nq_[:, None]).astype(NPF8)
                        oht[lane, slotq, dq] = 1.0
                        iflat[slotq, lane] = rq.astype(np.int16)
        idx_slab = np.zeros((16, cfg.IDX_COLS), np.int16)
        for g in range(len(cfg.groups)):
            for q in range(NQ):
                t0 = cfg.grp_slot_base[g] + cfg.grp_q_off[g][q]
                nq = cfg.grp_q_n[g][q]
                if nq == 0:
                    continue
                stream = iflat[t0: t0 + nq].reshape(-1)
                col0 = cfg.grp_idx_col[g][q]
                idx_slab[:, col0: col0 + len(stream) // 16] = (
                    stream.reshape(-1, 16).T)

        dinv_w = np.zeros((P, W), np.float32)
        gloc = np.full((P, W), -1.0, np.float32)
        base_n = c * cfg.NSH
        for w in range(W):
            rows = min(P, cfg.NSH - w * P)
            dinv_w[:rows, w] = dinv[base_n + w * P: base_n + w * P + rows]
            gloc[:rows, w] = batch[base_n + w * P: base_n + w * P + rows] - glo[c]
        ohg = (gloc[:, :, None] == np.arange(NGM, dtype=np.float32)[None, None, :])
        ohg = ohg.astype(NPBF16).reshape(P, W * NGM)

        cnt_loc = np.ones((NGM, 1), np.float32)
        cnt_loc[: ng[c], 0] = cntinv[glo[c]: ghi[c] + 1]
        sel = np.zeros((NGM, 2), np.float32)
        sel[0, 0] = 1.0
        sel[ng[c] - 1, 1] = 1.0
        msel = np.zeros((16, NGM), np.float32)
        if c + 1 < NCORES and glo[c + 1] == ghi[c]:
            msel[2 * (c + 1) + 0, ng[c] - 1] = 1.0

        per_core.append({
            "atab": atab.reshape(P, cfg.SLOT_TOT * 16),
            "idxs": np.tile(idx_slab, (8, 1)),
            "oht": oht.reshape(P, cfg.SLOT_TOT * P),
            "ohgt": ohg,
            "dinv_w": dinv_w,
            "cntinv_loc": cnt_loc,
            "sel": sel,
            "msel": msel,
        })

    W1p = np.zeros((16, cfg.DH), NPBF16)
    W1p[: cfg.DIN] = np.asarray(inputs["W1"], np.float32).astype(NPBF16)
    W2 = np.asarray(inputs["W2"], np.float32)
    W2_sb = np.concatenate([W2[:P, :], W2[P:, :]], axis=1).astype(NPBF16)
    b1 = np.asarray(inputs["b1"], np.float32).reshape(2, P).T.copy()
    b2 = np.asarray(inputs["b2"], np.float32)
    b2b = np.tile(b2[None, :], (P, 1)).astype(np.float32)
    fc1 = np.asarray(inputs["fc1_w"], np.float32)
    fc1_sb = np.concatenate([fc1[:P, :], fc1[P:, :]], axis=1)
    fc1b_bc = np.tile(np.asarray(inputs["fc1_b"], np.float32)[None, :], (NGM, 1))
    fc2 = np.asarray(inputs["fc2_w"], np.float32)
    fc2b_bc = np.tile(np.asarray(inputs["fc2_b"], np.float32)[None, :], (NGM, 1))
    ident = np.eye(P, dtype=np.float32)

    shared = {
        "W1p": W1p,
        "W2_sb": W2_sb,
        "b1h": b1,
        "b2b": b2b,
        "fc1_sb": fc1_sb,
        "fc1b_bc": fc1b_bc,
        "fc2_sb": fc2,
        "fc2b_bc": fc2b_bc,
        "ident": ident,
        "has_b1": bool(np.any(b1)),
        "has_b2": bool(np.any(b2b)),
    }
    for im in per_core:
        for k, v in shared.items():
            if k not in ("has_b1", "has_b2"):
                im[k] = v
    return per_core, shared


def build_nc(cfg, has_b1, has_b2):
    nc = bacc.Bacc("TRN2", target_bir_lowering=False, debug=False,
                   num_devices=NCORES, num_swdge_queues=NQ)
    N, G, W = cfg.N, cfg.G, cfg.W
    DH, NGM = cfg.DH, cfg.NGMAX

    atab_d = nc.dram_tensor("atab", [P, cfg.SLOT_TOT * 16], F8, kind="ExternalInput")
    idxs = nc.dram_tensor("idxs", [P, cfg.IDX_COLS], I16, kind="ExternalInput")
    oht_d = nc.dram_tensor("oht", [P, cfg.SLOT_TOT * P], F8, kind="ExternalInput")
    dinv_d = nc.dram_tensor("dinv_w", [P, W], F32, kind="ExternalInput")
    ohgt_d = nc.dram_tensor("ohgt", [P, W * NGM], BF16, kind="ExternalInput")
    W1p_d = nc.dram_tensor("W1p", [16, DH], BF16, kind="ExternalInput")
    W2_d = nc.dram_tensor("W2_sb", [P, 2 * DH], BF16, kind="ExternalInput")
    b1_d = nc.dram_tensor("b1h", [P, 2], F32, kind="ExternalInput")
    b2b_d = nc.dram_tensor("b2b", [P, DH], F32, kind="ExternalInput")
    fc1_d = nc.dram_tensor("fc1_sb", [P, 2 * cfg.DFC], F32, kind="ExternalInput")
    fc1b_d = nc.dram_tensor("fc1b_bc", [NGM, cfg.DFC], F32, kind="ExternalInput")
    fc2_d = nc.dram_tensor("fc2_sb", [cfg.DFC, cfg.NCLS], F32, kind="ExternalInput")
    fc2b_d = nc.dram_tensor("fc2b_bc", [NGM, cfg.NCLS], F32, kind="ExternalInput")
    ident_d = nc.dram_tensor("ident", [P, P], F32, kind="ExternalInput")
    cnt_d = nc.dram_tensor("cntinv_loc", [NGM, 1], F32, kind="ExternalInput")
    sel_d = nc.dram_tensor("sel", [NGM, 2], F32, kind="ExternalInput")
    msel_d = nc.dram_tensor("msel", [16, NGM], F32, kind="ExternalInput")

    out_d = nc.dram_tensor("out", [NGM, cfg.NCLS], F32, kind="ExternalOutput")

    shard_a = nc.dram_tensor("shard_a", [cfg.ROWSA, DH], F8)
    shard_b = nc.dram_tensor("shard_b", [cfg.ROWSB, DH], F8)
    h2ta = nc.dram_tensor("h2ta", [cfg.NA, DH], F8, addr_space="Shared")
    h2tb = nc.dram_tensor("h2tb", [cfg.NB, DH], F8, addr_space="Shared")
    bpub = nc.dram_tensor("bpub", [2, DH], F32)
    ball = nc.dram_tensor("ball", [2 * NCORES, DH], F32, addr_space="Shared")

    la_tile = [None]

    def selu3(scal, vec, out_ap, z_ap, tmp_pool, shape):
        r1 = tmp_pool.tile(shape, F32, tag="selu_r1")
        e = tmp_pool.tile(shape, F32, tag="selu_e")
        r2 = tmp_pool.tile(shape, F32, tag="selu_r2")
        scal.activation(r1[:], z_ap, AF.Relu, scale=SELU_LAM)
        scal.activation(e[:], z_ap, AF.Exp)
        scal.activation(r2[:], e[:], AF.Relu, bias=la_tile[0][:shape[0], 0:1],
                        scale=-SELU_LA)
        vec.tensor_tensor(out_ap, r1[:], r2[:], OP.subtract)

    def agg_matmuls(psum_ap, stat3, mov3, runs):
        calls = []
        for (off, n) in runs:
            for t in range(0, n - 1, 2):
                calls.append((off + t, 2))
            if n % 2:
                calls.append((off + n - 1, 1))
        nb = len(calls)
        for i, (sl, cnt) in enumerate(calls):
            if cnt == 2:
                nc.tensor.matmul(
                    psum_ap, stat3[:, sl: sl + 2, :], mov3[:, sl: sl + 2, :],
                    start=(i == 0), stop=(i == nb - 1), perf_mode=DR,
                )
            else:
                nc.tensor.matmul(
                    psum_ap, stat3[:, sl, :], mov3[:, sl, :],
                    start=(i == 0), stop=(i == nb - 1),
                )

    with tile.TileContext(nc) as tc:
        with (
            tc.tile_pool(name="consts", bufs=1) as cpool,
            tc.tile_pool(name="idxpool", bufs=1) as ipool,
            tc.tile_pool(name="atab", bufs=2) as apool,
            tc.tile_pool(name="gx2", bufs=5) as gx2pool,
            tc.tile_pool(name="oh", bufs=4) as ohpool,
            tc.tile_pool(name="work", bufs=3) as wpool,
            tc.tile_pool(name="head", bufs=1) as hpool,
            tc.tile_pool(name="post", bufs=2) as ppool,
            tc.tile_pool(name="ps_sm", bufs=2, space="PSUM") as ps_sm,
            tc.tile_pool(name="ps_h1", bufs=2, space="PSUM") as ps_h1,
            tc.tile_pool(name="ps_h2", bufs=2, space="PSUM") as ps_h2,
            tc.tile_pool(name="ps_pool", bufs=1, space="PSUM") as ps_pool,
        ):
            def load(pool, dram, shape, dt):
                t = pool.tile(shape, dt, tag=dram.name + "_sb")
                nc.sync.dma_start(out=t[:], in_=dram[tuple(slice(0, s) for s in shape)])
                return t

            la = cpool.tile([P, 1], F32, tag="la_const")
            nc.vector.memset(la[:], SELU_LA)
            la_tile[0] = la

            idx_sb = load(ipool, idxs, [P, cfg.IDX_COLS], I16)
            dinv_sb = load(cpool, dinv_d, [P, W], F32)
            W1p_sb = load(cpool, W1p_d, [16, DH], BF16)
            W2_sb = load(cpool, W2_d, [P, 2 * DH], BF16)
            b1_sb = load(cpool, b1_d, [P, 2], F32) if has_b1 else None
            b2b_sb = load(cpool, b2b_d, [P, DH], F32) if has_b2 else None
            fc1_sb = load(cpool, fc1_d, [P, 2 * cfg.DFC], F32)
            fc1b_sb = load(cpool, fc1b_d, [NGM, cfg.DFC], F32)
            fc2_sb = load(cpool, fc2_d, [cfg.DFC, cfg.NCLS], F32)
            fc2b_sb = load(cpool, fc2b_d, [NGM, cfg.NCLS], F32)
            ident_sb = load(cpool, ident_d, [P, P], F32)
            cnt_sb = load(cpool, cnt_d, [NGM, 1], F32)
            sel_sb = load(cpool, sel_d, [NGM, 2], F32)
            msel_sb = load(cpool, msel_d, [16, NGM], F32)

            def load_onehots(g):
                base = cfg.grp_slot_base[g]
                ns = cfg.grp_nslots[g]
                ohsl = ohpool.tile([P, ns, P], F8, tag="ohslab")
                nc.sync.dma_start(
                    out=ohsl[:], in_=oht_d[:, base * P: (base + ns) * P])
                return ohsl

            # ================= Phase A ======================================
            chunk_i = 0
            for g, (w0, wg) in enumerate(cfg.groups):
                base = cfg.grp_slot_base[g]
                ns = cfg.grp_nslots[g]
                at = apool.tile([P, ns, 16], F8, tag="atab_t")
                nc.sync.dma_start(out=at[:], in_=atab_d[:, base * 16: (base + ns) * 16])
                ohsl = load_onehots(g)
                for k in range(wg):
                    w = w0 + k
                    runs = _win_runs(cfg, g, k)
                    psA = ps_sm.tile([16, P], F32, tag="sm")
                    agg_matmuls(psA[:], at, ohsl, [(o, n) for (o, n) in runs])
                    aggT = wpool.tile([16, P], BF16, tag="aggT")
                    nc.scalar.copy(aggT[:], psA[:])
                    ph1 = ps_h1.tile([P, DH], F32, tag="ph1")
                    for j in range(2):
                        nc.tensor.matmul(
                            ph1[:, j * P: (j + 1) * P],
                            W1p_sb[:, j * P: (j + 1) * P], aggT[:],
                            start=True, stop=True,
                        )
                    h1T = ppool.tile([P, DH], BF16, tag="a_h1T")
                    if has_b1:
                        r1 = ppool.tile([P, DH], F32, tag="a_r1")
                        e = ppool.tile([P, DH], F32, tag="a_e")
                        r2 = ppool.tile([P, DH], F32, tag="a_r2")
                        for j in range(2):
                            sl_ = slice(j * P, (j + 1) * P)
                            nc.scalar.activation(r1[:, sl_], ph1[:, sl_], AF.Relu,
                                                 bias=b1_sb[:, j: j + 1],
                                                 scale=SELU_LAM)
                            nc.scalar.activation(e[:, sl_], ph1[:, sl_], AF.Exp,
                                                 bias=b1_sb[:, j: j + 1])
                        nc.scalar.activation(r2[:], e[:], AF.Relu,
                                             bias=la_tile[0][:, 0:1],
                                             scale=-SELU_LA)
                        nc.vector.tensor_tensor(h1T[:], r1[:], r2[:], OP.subtract)
                    else:
                        selu3(nc.scalar, nc.vector, h1T[:], ph1[:], ppool, [P, DH])

                    psum_h2t = ps_h2.tile([P, DH], F32, tag="main")
                    for j in range(2):
                        nc.tensor.matmul(
                            psum_h2t[:], h1T[:, j * P: (j + 1) * P],
                            W2_sb[:, j * DH: (j + 1) * DH],
                            start=(j == 0), stop=(j == 1),
                        )
                    h2tw = ppool.tile([P, DH], F8, tag="h2tw")
                    nc.scalar.activation(h2tw[:], psum_h2t[:], AF.Copy,
                                         scale=dinv_sb[:, w: w + 1])
                    rows = min(P, cfg.NSH - w * P)
                    if w < cfg.WA:
                        nc.sync.dma_start(out=shard_a[w * P: w * P + rows, :],
                                          in_=h2tw[:rows, :])
                    else:
                        r0 = w * P - cfg.ROWSA
                        nc.sync.dma_start(out=shard_b[r0: r0 + rows, :],
                                          in_=h2tw[:rows, :])
                # fire collective chunks at their group boundaries
                while (chunk_i < len(cfg.chunk_end_w)
                       and w0 + wg == cfg.chunk_end_w[chunk_i]):
                    half, lo, hi, brow = cfg.chunks[chunk_i]
                    shard = shard_a if half == 0 else shard_b
                    tabl = h2ta if half == 0 else h2tb
                    nrows = hi - lo
                    nc.gpsimd.collective_compute(
                        "AllGather", OP.bypass,
                        replica_groups=[list(range(NCORES))],
                        ins=[shard[lo:hi, :]],
                        outs=[tabl[brow: brow + NCORES * nrows, :]],
                    )
                    chunk_i += 1

            # ================= Phase B ======================================
            ppg = ps_pool.tile([NGM, DH], F32, tag="ppg")
            qflat = [h2ta[:, :], h2tb[:, :]]
            for g, (w0, wg) in enumerate(cfg.groups):
                base = cfg.grp_slot_base[g]
                ns = cfg.grp_nslots[g]
                gt2 = gx2pool.tile([P, ns, DH], F8, tag="gx2_t")
                for q in range(NQ):
                    nq = cfg.grp_q_n[g][q]
                    if nq == 0:
                        continue
                    s0 = cfg.grp_q_off[g][q]
                    nc.gpsimd.dma_gather(
                        gt2[:, s0: s0 + nq, :],
                        qflat[q // 2],
                        idx_sb[:, cfg.grp_idx_col[g][q]:
                               cfg.grp_idx_col[g][q] + nq * 8],
                        nq * P, nq * P, DH,
                        single_packet=False, queue_num=q,
                    )
                ohsl = load_onehots(g)
                ohg_sl = ohpool.tile([P, wg * NGM], BF16, tag="ohg_slab")
                nc.sync.dma_start(out=ohg_sl[:],
                                  in_=ohgt_d[:, w0 * NGM: (w0 + wg) * NGM])
                for k in range(wg):
                    w = w0 + k
                    runs = _win_runs(cfg, g, k)
                    psum2 = ps_h2.tile([P, DH], F32, tag="main")
                    agg_matmuls(psum2[:], ohsl, gt2, runs)
                    zd = ppool.tile([P, DH], F32, tag="b_zd")
                    nc.scalar.activation(zd[:], psum2[:], AF.Copy,
                                         scale=dinv_sb[:, w: w + 1])
                    if has_b2:
                        zb2 = ppool.tile([P, DH], F32, tag="b_zb2")
                        nc.vector.tensor_tensor(zb2[:], zd[:], b2b_sb[:], OP.add)
                        zd = zb2
                    h2w = ppool.tile([P, DH], BF16, tag="b_h2w")
                    selu3(nc.scalar, nc.vector, h2w[:], zd[:], ppool, [P, DH])
                    nc.tensor.matmul(
                        ppg[:], ohg_sl[:, k * NGM: (k + 1) * NGM], h2w[:],
                        start=(w == 0), stop=(w == W - 1),
                    )

            # ================= pooled head (local graphs) ===================
            ppT = hpool.tile([NGM, DH], F32, tag="ppT")
            nc.scalar.copy(ppT[:], ppg[:])
            pspub = ps_sm.tile([2, DH], F32, tag="sm")
            nc.tensor.matmul(pspub[:], sel_sb[:], ppT[:], start=True, stop=True)
            pub = hpool.tile([2, DH], F32, tag="pub")
            nc.scalar.copy(pub[:], pspub[:])
            nc.sync.dma_start(out=bpub[:, :], in_=pub[:, :])
            nc.gpsimd.collective_compute(
                "AllGather", OP.bypass,
                replica_groups=[list(range(NCORES))],
                ins=[bpub[:, :]], outs=[ball[:, :]],
            )
            ball_sb = hpool.tile([2 * NCORES, DH], F32, tag="ball_sb")
            nc.sync.dma_start(out=ball_sb[:], in_=ball[:, :])
            psm = ps_sm.tile([NGM, DH], F32, tag="sm")
            nc.tensor.matmul(psm[:], msel_sb[:], ball_sb[:], start=True, stop=True)
            pfull = hpool.tile([NGM, DH], F32, tag="pfull")
            nc.vector.tensor_tensor(pfull[:], ppT[:], psm[:], OP.add)
            pm = hpool.tile([NGM, DH], F32, tag="pm")
            nc.scalar.activation(pm[:], pfull[:], AF.Copy, scale=cnt_sb[:, 0:1])
            gsel = hpool.tile([NGM, DH], F32, tag="gsel")
            selu3(nc.scalar, nc.vector, gsel[:], pm[:], hpool, [NGM, DH])

            gT = hpool.tile([P, 2 * NGM], F32, tag="gT")
            for j in range(2):
                psT = ps_sm.tile([P, NGM], F32, tag="sm")
                nc.tensor.transpose(psT[:, :], gsel[:, j * P: (j + 1) * P],
                                    ident_sb[0:NGM, 0:NGM])
                nc.scalar.copy(gT[:, j * NGM: (j + 1) * NGM], psT[:])
            psum_fc1 = ps_h2.tile([NGM, cfg.DFC], F32, tag="main")
            for j in range(2):
                nc.tensor.matmul(
                    psum_fc1[:], gT[:, j * NGM: (j + 1) * NGM],
                    fc1_sb[:, j * cfg.DFC: (j + 1) * cfg.DFC],
                    start=(j == 0), stop=(j == 1),
                )
            zf = hpool.tile([NGM, cfg.DFC], F32, tag="zf")
            nc.vector.tensor_tensor(zf[:], psum_fc1[:], fc1b_sb[:], OP.add)
            hsel = hpool.tile([NGM, cfg.DFC], F32, tag="hsel")
            selu3(nc.scalar, nc.vector, hsel[:], zf[:], hpool, [NGM, cfg.DFC])

            psT2 = ps_sm.tile([cfg.DFC, NGM], F32, tag="sm")
            nc.tensor.transpose(psT2[:], hsel[:], ident_sb[0:NGM, 0:NGM])
            hT = hpool.tile([cfg.DFC, NGM], F32, tag="hT")
            nc.scalar.copy(hT[:], psT2[:])
            psum_fc2 = ps_sm.tile([NGM, cfg.NCLS], F32, tag="sm")
            nc.tensor.matmul(psum_fc2[:], hT[:], fc2_sb[:], start=True, stop=True)
            lg = hpool.tile([NGM, cfg.NCLS], F32, tag="lg")
            nc.vector.tensor_tensor(lg[:], psum_fc2[:], fc2b_sb[:], OP.add)

            nm = hpool.tile([NGM, 1], F32, tag="nm")
            nc.vector.tensor_reduce(nm[:], lg[:], mybir.AxisListType.X, OP.max,
                                    negate=True)
            e4 = hpool.tile([NGM, cfg.NCLS], F32, tag="e4")
            nc.scalar.activation(e4[:], lg[:], AF.Exp, bias=nm[:, 0:1])
            s4 = hpool.tile([NGM, 1], F32, tag="s4")
            nc.vector.tensor_reduce(s4[:], e4[:], mybir.AxisListType.X, OP.add)
            ls = hpool.tile([NGM, 1], F32, tag="ls")
            nc.scalar.activation(ls[:], s4[:], AF.Ln)
            q_ = hpool.tile([NGM, 1], F32, tag="q")
            nc.vector.tensor_tensor(q_[:], nm[:], ls[:], OP.subtract)
            outj = hpool.tile([NGM, cfg.NCLS], F32, tag="outj")
            nc.vector.tensor_scalar(outj[:], lg[:], q_[:, 0:1], None, OP.add)
            nc.sync.dma_start(out=out_d[0:NGM, :], in_=outj[:, :])

    nc.compile()
    return nc


_CACHE = {}


def run_gcn(inputs, n_nodes, n_graphs, d_in=14, d_hid=256, d_fc=128, n_cls=2,
            grp=3, trace=False):
    cl = CfgLike(n_nodes, grp)
    s, d = sort_edges(inputs, n_nodes)
    th_cw, cut = compute_tile_budget(cl, s, d, n_nodes, n_nodes // NCORES)
    batch = np.asarray(inputs["batch"], np.int64)
    glo, ghi, ng = graph_ranges(batch, n_nodes, n_graphs)
    ngmax = max(ng)
    assert np.unique(batch).size == n_graphs, "empty graphs not supported"
    cfg = Cfg(n_nodes, n_graphs, d_in, d_hid, d_fc, n_cls, th_cw, grp, ngmax)
    per_core, shared = host_prep(inputs, cfg, s, d, cut)
    key = (n_nodes, n_graphs, grp, ngmax, shared["has_b1"], shared["has_b2"],
           tuple(tuple(t) for t in cfg.TH))
    if key not in _CACHE:
        _CACHE[key] = build_nc(cfg, shared["has_b1"], shared["has_b2"])
    nc = _CACHE[key]
    res = run_bass_kernel_spmd(nc, per_core, list(range(NCORES)), trace=trace)
    out = np.zeros((n_graphs, n_cls), np.float32)
    for c in range(NCORES):
        lo = glo[c] + (1 if c > 0 and glo[c] == ghi[c - 1] else 0)
        loc = lo - glo[c]
        rows = np.asarray(res.results[c]["out"])
        out[lo: ghi[c] + 1] = rows[loc: ghi[c] - glo[c] + 1]
    return out, res


def kernel(**inputs) -> np.ndarray:
    out, _ = run_gcn(
        inputs, n_nodes=50000, n_graphs=256,
        trace=bool(int(os.environ.get("GCN_TRACE", "0"))),
    )
    return out


# revision 26
# speedup vs baseline: 1.1158x; 1.1158x over previous
"""Trainium2 Bass kernel for a 2-layer GCN + global mean pool + MLP head.

Strategy (8 NeuronCores, SPMD), v4:
  - Nodes (and incident edges grouped by destination window) sharded across
    8 cores; each core owns N/8 destination nodes.
  - Layer 1 needs NO dynamic gather: gather indices are static (edge_index
    is a host input), so the host materializes per-edge source features
    (x[src] * dinv[src] * dinv[dst], 14 cols padded to 16, fp8) directly in
    gather-slot layout.  The kernel streams the slab with large static DMAs
    and aggregates per destination window with one-hot DoubleRow-fp8
    matmuls, producing aggT = (A_norm x)^T with no transpose step.
  - Layer 2's table h2t = (h1 @ W2) * dinv is computed on-chip in fp8 and
    AllGather-ed in FOUR chunks into two 3-D shared tables (each < 32768
    rows so int16 gather indices address it with a single base); chunks
    fire at group boundaries so most collective time overlaps phase A.
    Per-edge dma_gather spreads over FOUR SWDGE queues (two per table,
    tile-granular balanced split; descriptor generation runs concurrently
    per queue on separate GpSimd core pairs).
  - selu(z) = lam*Relu(z) - lam*alpha*Relu(1 - exp(z)): 3 scalar acts +
    one DVE subtract.
  - Batch ids are sorted, so each core's nodes cover a contiguous slice of
    graphs (boundary graphs shared with neighbours).  Pooling runs per
    core over its local graph range; a 16-row AllGather exchanges the two
    boundary partials, each core runs the MLP head for the graphs it owns,
    and the host assembles the output slices.
"""

import os
import numpy as np
import ml_dtypes

import concourse.bacc as bacc
import concourse.bass as bass
import concourse.mybir as mybir
import concourse.tile as tile
from concourse.bass_utils import run_bass_kernel_spmd

F32 = mybir.dt.float32
F8 = mybir.dt.float8e4
BF16 = mybir.dt.bfloat16
I16 = mybir.dt.int16
AF = mybir.ActivationFunctionType
OP = mybir.AluOpType
DR = mybir.MatmulPerfMode.DoubleRow
NPBF16 = ml_dtypes.bfloat16
NPF8 = ml_dtypes.float8_e4m3

SELU_LAM = 1.0507009873554805
SELU_ALPHA = 1.6732632423543772
SELU_LA = SELU_LAM * SELU_ALPHA

P = 128
NCORES = 8
NQ = 4          # SWDGE queues
PREP_AHEAD = 4  # gather-descriptor prep pipeline depth (ring-capacity bound)
WA_GRPS = 9     # table A = first WA_GRPS * grp windows (27 for grp=3)
CHUNK_GRPS = (5, 4, 4, 4)   # collective chunks in groups (A1 A2 | B1 B2)


def share_of(T, k, qpar):
    hi, lo = -(-T // 2), T // 2
    if k % 2 == 0:
        return hi if qpar == 0 else lo
    return lo if qpar == 0 else hi


class Cfg:
    def __init__(self, n_nodes, n_graphs, d_in, d_hid, d_fc, n_cls, th_cw, grp,
                 ngmax):
        assert n_nodes % NCORES == 0
        self.N = n_nodes
        self.G = n_graphs
        self.DIN = d_in
        self.DH = d_hid
        self.DFC = d_fc
        self.NCLS = n_cls
        self.NSH = n_nodes // NCORES
        self.W = -(-self.NSH // P)
        self.GRP = grp
        self.NGMAX = ngmax
        self.WA = WA_GRPS * grp
        self.ROWSA = min(self.WA * P, self.NSH)
        self.ROWSB = self.NSH - self.ROWSA
        self.NA = NCORES * self.ROWSA
        self.NB = NCORES * self.ROWSB
        assert self.NA < 32768 and self.NB < 32768
        self.chunks = chunk_geom(grp, self.NSH)
        self.groups = []
        w = 0
        while w < self.W:
            wg = min(grp, self.W - w)
            self.groups.append((w, wg))
            w += wg
        self.TH = []
        for (w0, wg) in self.groups:
            th = []
            for h in range(2):
                t = max(-(-int(th_cw[c, w, h]) // P)
                        for c in range(NCORES) for w in range(w0, w0 + wg))
                th.append(max(t, 1))
            self.TH.append(th)
        self.grp_slot_base = []
        self.grp_q_off = []
        self.grp_q_n = []
        self.grp_idx_col = []
        self.win_q_off = []
        slot = 0
        col = 0
        for g, (w0, wg) in enumerate(self.groups):
            self.grp_slot_base.append(slot)
            qoffs, qns, icols, woffs = [], [], [], []
            off = 0
            for q in range(NQ):
                h, par = q // 2, q % 2
                qoffs.append(off)
                icols.append(col)
                wo = []
                for k in range(wg):
                    wo.append(off)
                    off += share_of(self.TH[g][h], k, par)
                woffs.append(wo)
                nq = off - qoffs[-1]
                qns.append(nq)
                col += nq * 8
            self.grp_q_off.append(qoffs)
            self.grp_q_n.append(qns)
            self.grp_idx_col.append(icols)
            self.win_q_off.append(woffs)
            slot += off
        self.SLOT_TOT = slot
        self.IDX_COLS = col
        self.grp_nslots = [sum(self.grp_q_n[g]) for g in range(len(self.groups))]
        # collective chunk boundaries in windows
        cb = np.cumsum([0] + [c * grp for c in CHUNK_GRPS])
        assert cb[-1] == self.W or cb[-1] == len(self.groups) * grp
        self.chunk_end_w = [min(int(x), self.W) for x in cb[1:]]
        assert self.chunk_end_w[1] == self.WA


def _win_runs(cfg, g, k):
    runs = []
    for q in range(NQ):
        h, par = q // 2, q % 2
        n = share_of(cfg.TH[g][h], k, par)
        runs.append((cfg.win_q_off[g][q][k], n))
    return runs


def chunk_geom(grp, nsh):
    """Chunk row ranges per core: [(half, row_lo, row_hi, base_row), ...]
    with row_lo/row_hi relative to the half's shard.  The shared tables are
    laid out chunk-major (chunk, core, rows) so every chunk's AllGather
    output is a contiguous region."""
    ends = np.cumsum([c * grp for c in CHUNK_GRPS])
    ends = [min(int(e) * P, nsh) for e in ends]
    rowsa = ends[1]
    out = []
    base_a = base_b = 0
    lo = 0
    for ci, hi in enumerate(ends):
        if ci < 2:
            out.append((0, lo, hi, base_a))
            base_a += NCORES * (hi - lo)
        else:
            out.append((1, lo - rowsa, hi - rowsa, base_b))
            base_b += NCORES * (hi - lo)
        lo = hi
    return out


def edge_half_map(cfg, s):
    r = s // cfg.NSH
    i = s % cfg.NSH
    h = np.zeros(len(s), np.int64)
    rel = np.zeros(len(s), np.int64)
    for (half, lo, hi, base) in cfg.chunks:
        alo = lo + (cfg.ROWSA if half else 0)
        ahi = hi + (cfg.ROWSA if half else 0)
        m = (i >= alo) & (i < ahi)
        nrows = hi - lo
        h[m] = half
        rel[m] = base + r[m] * nrows + (i[m] - alo)
    return h, rel


def sort_edges(inputs, n_nodes):
    ei = np.asarray(inputs["edge_index"], np.int64)
    N = n_nodes
    src = np.concatenate([ei[0], np.arange(N, dtype=np.int64)])
    dst = np.concatenate([ei[1], np.arange(N, dtype=np.int64)])
    order = np.argsort(dst, kind="stable")
    return src[order], dst[order]


def compute_tile_budget(cfg_like, s, d, n_nodes, nsh):
    W = -(-nsh // P)
    bounds = [c * nsh + w * P for c in range(NCORES) for w in range(W)] + [n_nodes]
    cut = np.searchsorted(d, np.asarray(bounds))
    h, _ = edge_half_map(cfg_like, s)
    th_cw = np.zeros((NCORES, W, 2), np.int64)
    for i in range(NCORES * W):
        hs = h[cut[i]: cut[i + 1]]
        th_cw[i // W, i % W, 0] = int((hs == 0).sum())
        th_cw[i // W, i % W, 1] = int((hs == 1).sum())
    return th_cw, cut


class CfgLike:
    def __init__(self, n_nodes, grp):
        self.N = n_nodes
        self.NSH = n_nodes // NCORES
        self.W = -(-self.NSH // P)
        self.WA = WA_GRPS * grp
        self.ROWSA = min(self.WA * P, self.NSH)
        self.ROWSB = self.NSH - self.ROWSA
        self.chunks = chunk_geom(grp, self.NSH)


def graph_ranges(batch, n_nodes, n_graphs):
    """Per-core local graph ranges (batch is sorted)."""
    nsh = n_nodes // NCORES
    glo, ghi = [], []
    for c in range(NCORES):
        glo.append(int(batch[c * nsh]))
        ghi.append(int(batch[(c + 1) * nsh - 1]))
    ng = [ghi[c] - glo[c] + 1 for c in range(NCORES)]
    return glo, ghi, ng


def host_prep(inputs, cfg, s, d, cut):
    N, G = cfg.N, cfg.G
    x = np.asarray(inputs["x"], np.float64)
    batch = np.asarray(inputs["batch"], np.int64)

    deg = np.bincount(d, minlength=N).astype(np.float64)
    dinv = 1.0 / np.sqrt(np.maximum(deg, 1.0))
    norm = dinv[s] * dinv[d]
    hmap, rel = edge_half_map(cfg, s)

    W = cfg.W
    cnt = np.bincount(batch, minlength=G).astype(np.float32)
    cntinv = (1.0 / np.maximum(cnt, 1.0)).astype(np.float32)
    glo, ghi, ng = graph_ranges(batch, N, G)
    NGM = cfg.NGMAX

    per_core = []
    for c in range(NCORES):
        atab = np.zeros((P, cfg.SLOT_TOT, 16), NPF8)
        oht = np.zeros((P, cfg.SLOT_TOT, P), NPF8)
        iflat = np.zeros((cfg.SLOT_TOT, P), np.int16)
        for g, (w0, wg) in enumerate(cfg.groups):
            base = cfg.grp_slot_base[g]
            for k in range(wg):
                w = w0 + k
                a, b = cut[c * W + w], cut[c * W + w + 1]
                sw, dw = s[a:b], d[a:b]
                nw, rw, hw = norm[a:b], rel[a:b], hmap[a:b]
                dl = (dw - (c * cfg.NSH + w * P)).astype(np.int64)
                for h in range(2):
                    m = hw == h
                    se, de, ne, re = sw[m], dl[m], nw[m], rw[m]
                    n = len(se)
                    s0 = share_of(cfg.TH[g][h], k, 0) * P
                    for par in range(2):
                        q = h * 2 + par
                        t0 = base + cfg.win_q_off[g][q][k]
                        seq = slice(0, min(n, s0)) if par == 0 else slice(min(n, s0), n)
                        sq, dq, nq_, rq = se[seq], de[seq], ne[seq], re[seq]
                        j = np.arange(len(sq))
                        slotq = t0 + j // P
                        lane = j % P
                        atab[lane, slotq, : cfg.DIN] = (
                            x[sq] * nq_[:, None]).astype(NPF8)
                        oht[lane, slotq, dq] = 1.0
                        iflat[slotq, lane] = rq.astype(np.int16)
        idx_slab = np.zeros((16, cfg.IDX_COLS), np.int16)
        for g in range(len(cfg.groups)):
            for q in range(NQ):
                t0 = cfg.grp_slot_base[g] + cfg.grp_q_off[g][q]
                nq = cfg.grp_q_n[g][q]
                if nq == 0:
                    continue
                stream = iflat[t0: t0 + nq].reshape(-1)
                col0 = cfg.grp_idx_col[g][q]
                idx_slab[:, col0: col0 + len(stream) // 16] = (
                    stream.reshape(-1, 16).T)

        dinv_w = np.zeros((P, W), np.float32)
        gloc = np.full((P, W), -1.0, np.float32)
        base_n = c * cfg.NSH
        for w in range(W):
            rows = min(P, cfg.NSH - w * P)
            dinv_w[:rows, w] = dinv[base_n + w * P: base_n + w * P + rows]
            gloc[:rows, w] = batch[base_n + w * P: base_n + w * P + rows] - glo[c]
        ohg = (gloc[:, :, None] == np.arange(NGM, dtype=np.float32)[None, None, :])
        ohg = ohg.astype(NPBF16).reshape(P, W * NGM)

        cnt_loc = np.ones((NGM, 1), np.float32)
        cnt_loc[: ng[c], 0] = cntinv[glo[c]: ghi[c] + 1]
        sel = np.zeros((NGM, 2), np.float32)
        sel[0, 0] = 1.0
        sel[ng[c] - 1, 1] = 1.0
        msel = np.zeros((16, NGM), np.float32)
        if c + 1 < NCORES and glo[c + 1] == ghi[c]:
            msel[2 * (c + 1) + 0, ng[c] - 1] = 1.0

        per_core.append({
            "atab": atab.reshape(P, cfg.SLOT_TOT * 16),
            "idxs": np.tile(idx_slab, (8, 1)),
            "oht": oht.reshape(P, cfg.SLOT_TOT * P),
            "ohgt": ohg,
            "dinv_w": dinv_w,
            "cntinv_loc": cnt_loc,
            "sel": sel,
            "msel": msel,
        })

    W1p = np.zeros((16, cfg.DH), NPBF16)
    W1p[: cfg.DIN] = np.asarray(inputs["W1"], np.float32).astype(NPBF16)
    W2 = np.asarray(inputs["W2"], np.float32)
    W2_sb = np.concatenate([W2[:P, :], W2[P:, :]], axis=1).astype(NPBF16)
    b1 = np.asarray(inputs["b1"], np.float32).reshape(2, P).T.copy()
    b2 = np.asarray(inputs["b2"], np.float32)
    b2b = np.tile(b2[None, :], (P, 1)).astype(np.float32)
    fc1 = np.asarray(inputs["fc1_w"], np.float32)
    fc1_sb = np.concatenate([fc1[:P, :], fc1[P:, :]], axis=1)
    fc1b_bc = np.tile(np.asarray(inputs["fc1_b"], np.float32)[None, :], (NGM, 1))
    fc2 = np.asarray(inputs["fc2_w"], np.float32)
    fc2b_bc = np.tile(np.asarray(inputs["fc2_b"], np.float32)[None, :], (NGM, 1))
    ident = np.eye(P, dtype=np.float32)

    shared = {
        "W1p": W1p,
        "W2_sb": W2_sb,
        "b1h": b1,
        "b2b": b2b,
        "fc1_sb": fc1_sb,
        "fc1b_bc": fc1b_bc,
        "fc2_sb": fc2,
        "fc2b_bc": fc2b_bc,
        "ident": ident,
        "has_b1": bool(np.any(b1)),
        "has_b2": bool(np.any(b2b)),
    }
    for im in per_core:
        for k, v in shared.items():
            if k not in ("has_b1", "has_b2"):
                im[k] = v
    return per_core, shared


def build_nc(cfg, has_b1, has_b2):
    nc = bacc.Bacc("TRN2", target_bir_lowering=False, debug=False,
                   num_devices=NCORES, num_swdge_queues=NQ,
                   dynamic_dma_scratch_size=32768)
    N, G, W = cfg.N, cfg.G, cfg.W
    DH, NGM = cfg.DH, cfg.NGMAX

    atab_d = nc.dram_tensor("atab", [P, cfg.SLOT_TOT * 16], F8, kind="ExternalInput")
    idxs = nc.dram_tensor("idxs", [P, cfg.IDX_COLS], I16, kind="ExternalInput")
    oht_d = nc.dram_tensor("oht", [P, cfg.SLOT_TOT * P], F8, kind="ExternalInput")
    dinv_d = nc.dram_tensor("dinv_w", [P, W], F32, kind="ExternalInput")
    ohgt_d = nc.dram_tensor("ohgt", [P, W * NGM], BF16, kind="ExternalInput")
    W1p_d = nc.dram_tensor("W1p", [16, DH], BF16, kind="ExternalInput")
    W2_d = nc.dram_tensor("W2_sb", [P, 2 * DH], BF16, kind="ExternalInput")
    b1_d = nc.dram_tensor("b1h", [P, 2], F32, kind="ExternalInput")
    b2b_d = nc.dram_tensor("b2b", [P, DH], F32, kind="ExternalInput")
    fc1_d = nc.dram_tensor("fc1_sb", [P, 2 * cfg.DFC], F32, kind="ExternalInput")
    fc1b_d = nc.dram_tensor("fc1b_bc", [NGM, cfg.DFC], F32, kind="ExternalInput")
    fc2_d = nc.dram_tensor("fc2_sb", [cfg.DFC, cfg.NCLS], F32, kind="ExternalInput")
    fc2b_d = nc.dram_tensor("fc2b_bc", [NGM, cfg.NCLS], F32, kind="ExternalInput")
    ident_d = nc.dram_tensor("ident", [P, P], F32, kind="ExternalInput")
    cnt_d = nc.dram_tensor("cntinv_loc", [NGM, 1], F32, kind="ExternalInput")
    sel_d = nc.dram_tensor("sel", [NGM, 2], F32, kind="ExternalInput")
    msel_d = nc.dram_tensor("msel", [16, NGM], F32, kind="ExternalInput")

    out_d = nc.dram_tensor("out", [NGM, cfg.NCLS], F32, kind="ExternalOutput")

    shards = [nc.dram_tensor(f"shard_c{i}", [hi - lo, DH], F8)
              for i, (half, lo, hi, brow) in enumerate(cfg.chunks)]
    h2ta = nc.dram_tensor("h2ta", [cfg.NA, DH], F8, addr_space="Shared")
    h2tb = nc.dram_tensor("h2tb", [cfg.NB, DH], F8, addr_space="Shared")
    bpub = nc.dram_tensor("bpub", [2, DH], F32)
    ball = nc.dram_tensor("ball", [2 * NCORES, DH], F32, addr_space="Shared")

    prep_sems = [nc.alloc_semaphore(f"prep_q{q}") for q in range(NQ)]
    dma_sems = [nc.alloc_semaphore(f"gdma_q{q}") for q in range(NQ)]

    la_tile = [None]

    def selu3(scal, vec, out_ap, z_ap, tmp_pool, shape):
        r1 = tmp_pool.tile(shape, F32, tag="selu_r1")
        e = tmp_pool.tile(shape, F32, tag="selu_e")
        r2 = tmp_pool.tile(shape, F32, tag="selu_r2")
        scal.activation(r1[:], z_ap, AF.Relu, scale=SELU_LAM)
        scal.activation(e[:], z_ap, AF.Exp)
        scal.activation(r2[:], e[:], AF.Relu, bias=la_tile[0][:shape[0], 0:1],
                        scale=-SELU_LA)
        vec.tensor_tensor(out_ap, r1[:], r2[:], OP.subtract)

    def agg_matmuls(psum_ap, stat3, mov3, runs):
        calls = []
        for (off, n) in runs:
            for t in range(0, n - 1, 2):
                calls.append((off + t, 2))
            if n % 2:
                calls.append((off + n - 1, 1))
        nb = len(calls)
        for i, (sl, cnt) in enumerate(calls):
            if cnt == 2:
                nc.tensor.matmul(
                    psum_ap, stat3[:, sl: sl + 2, :], mov3[:, sl: sl + 2, :],
                    start=(i == 0), stop=(i == nb - 1), perf_mode=DR,
                )
            else:
                nc.tensor.matmul(
                    psum_ap, stat3[:, sl, :], mov3[:, sl, :],
                    start=(i == 0), stop=(i == nb - 1),
                )

    with tile.TileContext(nc) as tc:
        with (
            tc.tile_pool(name="consts", bufs=1) as cpool,
            tc.tile_pool(name="idxpool", bufs=1) as ipool,
            tc.tile_pool(name="atab", bufs=2) as apool,
            tc.tile_pool(name="gx2", bufs=5) as gx2pool,
            tc.tile_pool(name="oh", bufs=4) as ohpool,
            tc.tile_pool(name="work", bufs=3) as wpool,
            tc.tile_pool(name="head", bufs=1) as hpool,
            tc.tile_pool(name="post", bufs=2) as ppool,
            tc.tile_pool(name="ps_sm", bufs=2, space="PSUM") as ps_sm,
            tc.tile_pool(name="ps_h1", bufs=2, space="PSUM") as ps_h1,
            tc.tile_pool(name="ps_h2", bufs=2, space="PSUM") as ps_h2,
            tc.tile_pool(name="ps_pool", bufs=1, space="PSUM") as ps_pool,
        ):
            def load(pool, dram, shape, dt):
                t = pool.tile(shape, dt, tag=dram.name + "_sb")
                nc.sync.dma_start(out=t[:], in_=dram[tuple(slice(0, s) for s in shape)])
                return t

            la = cpool.tile([P, 1], F32, tag="la_const")
            nc.vector.memset(la[:], SELU_LA)
            la_tile[0] = la

            idx_sb = load(ipool, idxs, [P, cfg.IDX_COLS], I16)
            dinv_sb = load(cpool, dinv_d, [P, W], F32)
            W1p_sb = load(cpool, W1p_d, [16, DH], BF16)
            W2_sb = load(cpool, W2_d, [P, 2 * DH], BF16)
            b1_sb = load(cpool, b1_d, [P, 2], F32) if has_b1 else None
            b2b_sb = load(cpool, b2b_d, [P, DH], F32) if has_b2 else None
            fc1_sb = load(cpool, fc1_d, [P, 2 * cfg.DFC], F32)
            fc1b_sb = load(cpool, fc1b_d, [NGM, cfg.DFC], F32)
            fc2_sb = load(cpool, fc2_d, [cfg.DFC, cfg.NCLS], F32)
            fc2b_sb = load(cpool, fc2b_d, [NGM, cfg.NCLS], F32)
            ident_sb = load(cpool, ident_d, [P, P], F32)
            cnt_sb = load(cpool, cnt_d, [NGM, 1], F32)
            sel_sb = load(cpool, sel_d, [NGM, 2], F32)
            msel_sb = load(cpool, msel_d, [16, NGM], F32)

            def load_onehots(g):
                base = cfg.grp_slot_base[g]
                ns = cfg.grp_nslots[g]
                ohsl = ohpool.tile([P, ns, P], F8, tag="ohslab")
                nc.sync.dma_start(
                    out=ohsl[:], in_=oht_d[:, base * P: (base + ns) * P])
                return ohsl

            # ================= Phase A ======================================
            chunk_i = 0
            for g, (w0, wg) in enumerate(cfg.groups):
                base = cfg.grp_slot_base[g]
                ns = cfg.grp_nslots[g]
                at = apool.tile([P, ns, 16], F8, tag="atab_t")
                nc.sync.dma_start(out=at[:], in_=atab_d[:, base * 16: (base + ns) * 16])
                ohsl = load_onehots(g)
                for k in range(wg):
                    w = w0 + k
                    runs = _win_runs(cfg, g, k)
                    psA = ps_sm.tile([16, P], F32, tag="sm")
                    agg_matmuls(psA[:], at, ohsl, [(o, n) for (o, n) in runs])
                    aggT = wpool.tile([16, P], BF16, tag="aggT")
                    nc.scalar.copy(aggT[:], psA[:])
                    ph1 = ps_h1.tile([P, DH], F32, tag="ph1")
                    for j in range(2):
                        nc.tensor.matmul(
                            ph1[:, j * P: (j + 1) * P],
                            W1p_sb[:, j * P: (j + 1) * P], aggT[:],
                            start=True, stop=True,
                        )
                    h1T = ppool.tile([P, DH], BF16, tag="a_h1T")
                    if has_b1:
                        r1 = ppool.tile([P, DH], F32, tag="a_r1")
                        e = ppool.tile([P, DH], F32, tag="a_e")
                        r2 = ppool.tile([P, DH], F32, tag="a_r2")
                        for j in range(2):
                            sl_ = slice(j * P, (j + 1) * P)
                            nc.scalar.activation(r1[:, sl_], ph1[:, sl_], AF.Relu,
                                                 bias=b1_sb[:, j: j + 1],
                                                 scale=SELU_LAM)
                            nc.scalar.activation(e[:, sl_], ph1[:, sl_], AF.Exp,
                                                 bias=b1_sb[:, j: j + 1])
                        nc.scalar.activation(r2[:], e[:], AF.Relu,
                                             bias=la_tile[0][:, 0:1],
                                             scale=-SELU_LA)
                        nc.vector.tensor_tensor(h1T[:], r1[:], r2[:], OP.subtract)
                    else:
                        selu3(nc.scalar, nc.vector, h1T[:], ph1[:], ppool, [P, DH])

                    psum_h2t = ps_h2.tile([P, DH], F32, tag="main")
                    for j in range(2):
                        nc.tensor.matmul(
                            psum_h2t[:], h1T[:, j * P: (j + 1) * P],
                            W2_sb[:, j * DH: (j + 1) * DH],
                            start=(j == 0), stop=(j == 1),
                        )
                    h2tw = ppool.tile([P, DH], F8, tag="h2tw")
                    nc.scalar.activation(h2tw[:], psum_h2t[:], AF.Copy,
                                         scale=dinv_sb[:, w: w + 1])
                    rows = min(P, cfg.NSH - w * P)
                    nrow = w * P  # shard-local node row
                    for ci, (half, clo, chi, brow) in enumerate(cfg.chunks):
                        alo = clo + (cfg.ROWSA if half else 0)
                        ahi = chi + (cfg.ROWSA if half else 0)
                        if alo <= nrow < ahi:
                            nc.sync.dma_start(
                                out=shards[ci][nrow - alo: nrow - alo + rows, :],
                                in_=h2tw[:rows, :])
                            break
                # fire collective chunks at their group boundaries
                while (chunk_i < len(cfg.chunk_end_w)
                       and w0 + wg == cfg.chunk_end_w[chunk_i]):
                    half, lo, hi, brow = cfg.chunks[chunk_i]
                    tabl = h2ta if half == 0 else h2tb
                    nrows = hi - lo
                    nc.gpsimd.collective_compute(
                        "AllGather", OP.bypass,
                        replica_groups=[list(range(NCORES))],
                        ins=[shards[chunk_i][:, :]],
                        outs=[tabl[brow: brow + NCORES * nrows, :]],
                    )
                    chunk_i += 1

            # ================= Phase B ======================================
            ppg = ps_pool.tile([NGM, DH], F32, tag="ppg")
            qflat = [h2ta[:, :], h2tb[:, :]]
            for g, (w0, wg) in enumerate(cfg.groups):
                base = cfg.grp_slot_base[g]
                ns = cfg.grp_nslots[g]
                gt2 = gx2pool.tile([P, ns, DH], F8, tag="gx2_t")
                for q in range(NQ):
                    nq = cfg.grp_q_n[g][q]
                    if nq == 0:
                        continue
                    s0 = cfg.grp_q_off[g][q]
                    nc.gpsimd.dma_gather(
                        gt2[:, s0: s0 + nq, :],
                        qflat[q // 2],
                        idx_sb[:, cfg.grp_idx_col[g][q]:
                               cfg.grp_idx_col[g][q] + nq * 8],
                        nq * P, nq * P, DH,
                        single_packet=False, queue_num=q,
                    )
                ohsl = load_onehots(g)
                ohg_sl = ohpool.tile([P, wg * NGM], BF16, tag="ohg_slab")
                nc.sync.dma_start(out=ohg_sl[:],
                                  in_=ohgt_d[:, w0 * NGM: (w0 + wg) * NGM])
                for k in range(wg):
                    w = w0 + k
                    runs = _win_runs(cfg, g, k)
                    psum2 = ps_h2.tile([P, DH], F32, tag="main")
                    agg_matmuls(psum2[:], ohsl, gt2, runs)
                    zd = ppool.tile([P, DH], F32, tag="b_zd")
                    nc.scalar.activation(zd[:], psum2[:], AF.Copy,
                                         scale=dinv_sb[:, w: w + 1])
                    if has_b2:
                        zb2 = ppool.tile([P, DH], F32, tag="b_zb2")
                        nc.vector.tensor_tensor(zb2[:], zd[:], b2b_sb[:], OP.add)
                        zd = zb2
                    h2w = ppool.tile([P, DH], BF16, tag="b_h2w")
                    selu3(nc.scalar, nc.vector, h2w[:], zd[:], ppool, [P, DH])
                    nc.tensor.matmul(
                        ppg[:], ohg_sl[:, k * NGM: (k + 1) * NGM], h2w[:],
                        start=(w == 0), stop=(w == W - 1),
                    )

            # ================= pooled head (local graphs) ===================
            ppT = hpool.tile([NGM, DH], F32, tag="ppT")
            nc.scalar.copy(ppT[:], ppg[:])
            pspub = ps_sm.tile([2, DH], F32, tag="sm")
            nc.tensor.matmul(pspub[:], sel_sb[:], ppT[:], start=True, stop=True)
            pub = hpool.tile([2, DH], F32, tag="pub")
            nc.scalar.copy(pub[:], pspub[:])
            nc.sync.dma_start(out=bpub[:, :], in_=pub[:, :])
            nc.gpsimd.collective_compute(
                "AllGather", OP.bypass,
                replica_groups=[list(range(NCORES))],
                ins=[bpub[:, :]], outs=[ball[:, :]],
            )
            ball_sb = hpool.tile([2 * NCORES, DH], F32, tag="ball_sb")
            nc.sync.dma_start(out=ball_sb[:], in_=ball[:, :])
            psm = ps_sm.tile([NGM, DH], F32, tag="sm")
            nc.tensor.matmul(psm[:], msel_sb[:], ball_sb[:], start=True, stop=True)
            pfull = hpool.tile([NGM, DH], F32, tag="pfull")
            nc.vector.tensor_tensor(pfull[:], ppT[:], psm[:], OP.add)
            pm = hpool.tile([NGM, DH], F32, tag="pm")
            nc.scalar.activation(pm[:], pfull[:], AF.Copy, scale=cnt_sb[:, 0:1])
            gsel = hpool.tile([NGM, DH], F32, tag="gsel")
            selu3(nc.scalar, nc.vector, gsel[:], pm[:], hpool, [NGM, DH])

            gT = hpool.tile([P, 2 * NGM], F32, tag="gT")
            for j in range(2):
                psT = ps_sm.tile([P, NGM], F32, tag="sm")
                nc.tensor.transpose(psT[:, :], gsel[:, j * P: (j + 1) * P],
                                    ident_sb[0:NGM, 0:NGM])
                nc.scalar.copy(gT[:, j * NGM: (j + 1) * NGM], psT[:])
            psum_fc1 = ps_h2.tile([NGM, cfg.DFC], F32, tag="main")
            for j in range(2):
                nc.tensor.matmul(
                    psum_fc1[:], gT[:, j * NGM: (j + 1) * NGM],
                    fc1_sb[:, j * cfg.DFC: (j + 1) * cfg.DFC],
                    start=(j == 0), stop=(j == 1),
                )
            zf = hpool.tile([NGM, cfg.DFC], F32, tag="zf")
            nc.vector.tensor_tensor(zf[:], psum_fc1[:], fc1b_sb[:], OP.add)
            hsel = hpool.tile([NGM, cfg.DFC], F32, tag="hsel")
            selu3(nc.scalar, nc.vector, hsel[:], zf[:], hpool, [NGM, cfg.DFC])

            psT2 = ps_sm.tile([cfg.DFC, NGM], F32, tag="sm")
            nc.tensor.transpose(psT2[:], hsel[:], ident_sb[0:NGM, 0:NGM])
            hT = hpool.tile([cfg.DFC, NGM], F32, tag="hT")
            nc.scalar.copy(hT[:], psT2[:])
            psum_fc2 = ps_sm.tile([NGM, cfg.NCLS], F32, tag="sm")
            nc.tensor.matmul(psum_fc2[:], hT[:], fc2_sb[:], start=True, stop=True)
            lg = hpool.tile([NGM, cfg.NCLS], F32, tag="lg")
            nc.vector.tensor_tensor(lg[:], psum_fc2[:], fc2b_sb[:], OP.add)

            nm = hpool.tile([NGM, 1], F32, tag="nm")
            nc.vector.tensor_reduce(nm[:], lg[:], mybir.AxisListType.X, OP.max,
                                    negate=True)
            e4 = hpool.tile([NGM, cfg.NCLS], F32, tag="e4")
            nc.scalar.activation(e4[:], lg[:], AF.Exp, bias=nm[:, 0:1])
            s4 = hpool.tile([NGM, 1], F32, tag="s4")
            nc.vector.tensor_reduce(s4[:], e4[:], mybir.AxisListType.X, OP.add)
            ls = hpool.tile([NGM, 1], F32, tag="ls")
            nc.scalar.activation(ls[:], s4[:], AF.Ln)
            q_ = hpool.tile([NGM, 1], F32, tag="q")
            nc.vector.tensor_tensor(q_[:], nm[:], ls[:], OP.subtract)
            outj = hpool.tile([NGM, cfg.NCLS], F32, tag="outj")
            nc.vector.tensor_scalar(outj[:], lg[:], q_[:, 0:1], None, OP.add)
            nc.sync.dma_start(out=out_d[0:NGM, :], in_=outj[:, :])

    nc.compile()
    return nc


_CACHE = {}


def run_gcn(inputs, n_nodes, n_graphs, d_in=14, d_hid=256, d_fc=128, n_cls=2,
            grp=3, trace=False):
    cl = CfgLike(n_nodes, grp)
    s, d = sort_edges(inputs, n_nodes)
    th_cw, cut = compute_tile_budget(cl, s, d, n_nodes, n_nodes // NCORES)
    batch = np.asarray(inputs["batch"], np.int64)
    glo, ghi, ng = graph_ranges(batch, n_nodes, n_graphs)
    ngmax = max(ng)
    assert np.unique(batch).size == n_graphs, "empty graphs not supported"
    cfg = Cfg(n_nodes, n_graphs, d_in, d_hid, d_fc, n_cls, th_cw, grp, ngmax)
    per_core, shared = host_prep(inputs, cfg, s, d, cut)
    key = (n_nodes, n_graphs, grp, ngmax, shared["has_b1"], shared["has_b2"],
           tuple(tuple(t) for t in cfg.TH))
    if key not in _CACHE:
        _CACHE[key] = build_nc(cfg, shared["has_b1"], shared["has_b2"])
    nc = _CACHE[key]
    res = run_bass_kernel_spmd(nc, per_core, list(range(NCORES)), trace=trace)
    out = np.zeros((n_graphs, n_cls), np.float32)
    for c in range(NCORES):
        lo = glo[c] + (1 if c > 0 and glo[c] == ghi[c - 1] else 0)
        loc = lo - glo[c]
        rows = np.asarray(res.results[c]["out"])
        out[lo: ghi[c] + 1] = rows[loc: ghi[c] - glo[c] + 1]
    return out, res


def kernel(**inputs) -> np.ndarray:
    out, _ = run_gcn(
        inputs, n_nodes=50000, n_graphs=256,
        trace=bool(int(os.environ.get("GCN_TRACE", "0"))),
    )
    return out
